# revision 10
# baseline (speedup 1.0000x reference)
"""CapsuleLayer (dynamic routing, 3 iterations) Trainium2 Bass kernel. v2

Full inputs:  input_vectors [32, 2048, 16] f32, weight_matrix [1, 64, 32, 16] f32
Full output:  [32, 64, 32] f32

Sharding: data-parallel over batch; each of 8 NeuronCores processes 4 batches.
weight-derived constants are replicated. No collectives.

Algorithm restructuring (never materializes u = [B,N,O,D] = 537MB):
  xs       = squash(x)                       (per-row scale g = n2/((eps+n2)(1e-8+n)))
  iter 0:  c uniform -> t0[o,i] = (1/64) sum_n xs[n,i]        (ones matmul)
  iter k:  logits = xs @ wv_sum.T            (bf16 matmul, K=16, row-tiled quads)
           e = exp(logits); Z = sum_o e; xz = xs / Z
           t[o,i] = sum_n e[n,o] * xz[n,i]   (bf16 matmul, K=128, 2-batch packed)
  wv      = h * (M2 @ t),  M2 = W^T W (host-precomputed Gram),  h = squash scale of s
            (uses n2 = ||s||^2 = t . (M2 @ t) so s itself is only built at the end)
  output  v = h * (W @ t)  at the last iteration.
Iteration 2 logits use rhs wv0+wv1 (linearity) so no cross-iteration PSUM state.

v2 performance changes vs v1:
  - n-mapping n = p*16 + j (was g*128 + p): input DMA descriptors become 1KB
    contiguous chunks instead of 64B (the indexing of every on-chip array is
    identical under the relabel g<->j; only the input DMA rearrange changes).
  - sqrt computed as exp(0.5*ln(n2)): every scalar-engine function used
    (Exp/Ln/Square/Copy) lives in the single natural_log_exp_and_others ACT
    table set -> no mid-kernel ~2.7us ACT_TABLE_LOAD swaps.
  - squash scale multiply writes the padded/permuted bf16 layout (xsb == old
    xsp) directly; xs f32 and the 4 repack copies are gone.
  - all 16 [128,128] SBUF transposes issued as ONE dma_start_transpose with a
    3D out AP (16 serialized Sync-queue instructions -> 1).
  - e, xz in bf16 (2x DVE read rate, FWL weight loads); t-matmul packs 2
    batches per instruction pair (lhsT [128,128] = [e_b0|e_b1]) halving the
    instruction count; off-diagonal quadrants of the [128,32] PSUM out are
    ignored.
  - small stage (q, n2, h, wv) runs once on both batch-pairs merged [128,32].
  - HAM warmup: a burst of dummy matmuls during the input DMA/squash phase
    keeps the PE clock at 2.4 GHz for the real work.
"""

import os

os.environ.setdefault("MYCRO_LOCAL_CACHE", "1")

import numpy as np
import ml_dtypes

import concourse.bass as bass
import concourse.tile as tile
from concourse import bacc, mybir
from concourse.bass_utils import run_bass_kernel_spmd

AF = mybir.ActivationFunctionType
ALU = mybir.AluOpType
F32 = mybir.dt.float32
BF16 = mybir.dt.bfloat16

N_CORES = 8
B = 4          # batches per core
N = 2048       # input capsules
O = 64         # output capsules
DI = 16        # input capsule dim
D = 32         # output capsule dim
G = 16         # n-groups per batch (n = p*16 + g)
EPS = 0.5

NWARM = int(os.environ.get("CAPS_NWARM", "40"))


def _strip(b, g):
    """(row_base, col_base) of the xsT strip for (batch b, n-group g).

    Quad layout: 4 concurrent K=16 agreement matmuls sit at row groups
    0/32/64/96 = (b%2)*64 + (g//8)*32; col block = ((b//2)*8 + (g%8))*128.
    """
    r = (b % 2) * 64 + (g // 8) * 32
    c = ((b // 2) * 8 + (g % 8)) * 128
    return r, c


def build_kernel(nc: bass.Bass, tc: tile.TileContext):
    from contextlib import ExitStack
    ctx = ExitStack()
    x = nc.dram_tensor("x", [B, N, DI], F32, kind="ExternalInput").ap()
    wrep = nc.dram_tensor("wrep", [128, D * DI], F32, kind="ExternalInput").ap()
    m2rep = nc.dram_tensor("m2rep", [128, DI * DI], F32, kind="ExternalInput").ap()
    ident = nc.dram_tensor("ident", [128, 128], BF16, kind="ExternalInput").ap()
    vout = nc.dram_tensor("vout", [B, O, D], F32, kind="ExternalOutput").ap()

    const = ctx.enter_context(tc.tile_pool(name="const", bufs=1))
    big = ctx.enter_context(tc.tile_pool(name="big", bufs=1))
    small = ctx.enter_context(tc.tile_pool(name="small", bufs=2))
    psum = ctx.enter_context(tc.tile_pool(name="psum", bufs=2, space="PSUM"))
    psum1 = ctx.enter_context(tc.tile_pool(name="psum1", bufs=1, space="PSUM"))

    # ---- constants ----
    w_sb = const.tile([128, D * DI], F32, tag="w_sb")
    m2_sb = const.tile([128, DI * DI], F32, tag="m2_sb")
    id_sb = const.tile([128, 128], BF16, tag="id_sb")
    ones128 = const.tile([128, 128], BF16, tag="ones128")
    nc.sync.dma_start(w_sb[:], wrep)
    nc.sync.dma_start(m2_sb[:], m2rep)
    nc.sync.dma_start(id_sb[:], ident)
    nc.gpsimd.memset(ones128[:], 1.0 / O)

    # persistent wv tiles: [128=(bl,o), 32=(i|pad)] pads stay zero forever
    wv_bf = [const.tile([128, 2 * DI], BF16, tag=f"wv_bf{P}", name=f"wv_bf{P}") for P in range(2)]
    for P in range(2):
        nc.gpsimd.memset(wv_bf[P][:], 0.0)
    wv0f = [const.tile([128, DI], F32, tag=f"wv0f_{P}", name=f"wv0f_{P}") for P in range(2)]

    # xsb: squashed input, bf16, padded strip layout
    #   col = block*128 + (b%2)*64 + (g//8)*32 + i,  block = (b//2)*8 + (g%8)
    # (cols 16..31 of each 32-strip are pad; memset once, early)
    xsb = big.tile([128, 2048], BF16, tag="xsb")
    nc.gpsimd.memset(xsb[:], 0.0)

    # ---- HAM warmup: dummy matmuls while DMA/squash run ----
    if NWARM:
        warm = psum1.tile([128, O], F32, tag="warm")
        for _ in range(NWARM):
            nc.tensor.matmul(warm[:], lhsT=id_sb[:], rhs=id_sb[:, :O],
                             start=True, stop=True)

    # ---- load x:  xr [128, (b, g, i)] with n = p*16 + g ----
    xr = big.tile([128, B * G * DI], F32, tag="xr")
    nc.sync.dma_start(
        xr[:].rearrange("p (b g i) -> p b g i", b=B, g=G),
        x.rearrange("b (p g) i -> p b g i", g=G),
    )

    # ---- squash ----
    xsq = big.tile([128, B * G * DI], F32, tag="xsq")
    nc.vector.tensor_mul(xsq[:], xr[:], xr[:])
    n2x = small.tile([128, B * G], F32, tag="n2x")
    nc.vector.reduce_sum(n2x[:], xsq[:].rearrange("p (r i) -> p r i", i=DI), axis=mybir.AxisListType.X)
    # n = sqrt(n2) = exp(0.5*ln(n2)); keeps scalar engine on one table set
    nx = small.tile([128, B * G], F32, tag="nx")
    nc.scalar.activation(nx[:], n2x[:], AF.Ln)
    nc.scalar.activation(nx[:], nx[:], AF.Exp, scale=0.5)
    nc.vector.tensor_scalar_add(nx[:], nx[:], 1e-8)
    denx = small.tile([128, B * G], F32, tag="denx")
    nc.vector.scalar_tensor_tensor(denx[:], n2x[:], EPS, nx[:], op0=ALU.add, op1=ALU.mult)
    nc.vector.reciprocal(denx[:], denx[:])
    gx = small.tile([128, B * G], F32, tag="gx")
    nc.vector.tensor_mul(gx[:], n2x[:], denx[:])

    # xsb views: per-batch strided [p, gl(8), gh(2), i(16)]
    def stripview(t):
        return t[:].rearrange("p (P gl bv gh c) -> p P gl bv gh c", P=2, gl=8, bv=2, gh=2)

    xsbv = stripview(xsb)
    for b in range(B):
        nc.vector.tensor_mul(
            xsbv[:, b // 2, :, b % 2, :, :DI],
            xr[:, b * G * DI:(b + 1) * G * DI].rearrange(
                "p (gh gl i) -> p gl gh i", gh=2, gl=8
            ),
            gx[:, b * G:(b + 1) * G].rearrange("p (gh gl) -> p gl gh", gh=2)
            .unsqueeze(3).broadcast_to([128, 8, 2, DI]),
        )

    # ---- xsT: all 16 block transposes in ONE instruction ----
    xsT = big.tile([128, 2048], BF16, tag="xsT")
    nc.sync.dma_start(
        xsT[:].rearrange("p (c f) -> p c f", f=128),
        xsb[:],
        transpose=True,
    )

    # ---- persistent state ----
    # e_sb cols: (P, g, bl, o)  -> lhsT pair = e_sb[:, (P*16+g)*128 : +128]
    e_sb = big.tile([128, 2 * G * 2 * O], BF16, tag="e_sb")
    rz = small.tile([128, B * G], F32, tag="rz")
    # xzb: same padded strip layout as xsb (pads never read)
    xzb = big.tile([128, 2048], BF16, tag="xzb")
    trc = [None, None]

    for it in range(3):
        if it > 0:
            for P in range(2):
                L = [psum.tile([128, G * O], F32, tag="logits", name=f"L{bv}") for bv in range(2)]
                # (bv, gh) vary fastest -> consecutive matmuls hit the 4
                # distinct PE row groups -> 4-way quad concurrency
                for gl in range(8):
                    for bv in range(2):
                        for gh in range(2):
                            b = 2 * P + bv
                            g = gh * 8 + gl
                            r, c = _strip(b, g)
                            nc.tensor.matmul(
                                L[bv][:, g * O:(g + 1) * O],
                                lhsT=xsT[r:r + DI, c:c + 128],
                                rhs=trc[P][r:r + DI, bv * O:(bv + 1) * O],
                                tile_position=(r, 0),
                                start=True,
                                stop=True,
                            )
                for bv in range(2):
                    b = 2 * P + bv
                    # e (bf16) strided into (P, g, bl, o) layout
                    ev = e_sb[:].rearrange(
                        "p (Pp g two o) -> p Pp g two o", Pp=2, g=G, two=2
                    )[:, P, :, bv, :]
                    nc.scalar.activation(ev, L[bv][:].rearrange("p (g o) -> p g o", o=O), AF.Exp)
                    zb = small.tile([128, G], F32, tag="zb")
                    nc.vector.reduce_sum(zb[:], ev, axis=mybir.AxisListType.X)
                    nc.vector.reciprocal(rz[:, b * G:(b + 1) * G], zb[:])
                    nc.vector.tensor_mul(
                        stripview(xzb)[:, b // 2, :, b % 2, :, :DI],
                        xsbv[:, b // 2, :, b % 2, :, :DI],
                        rz[:, b * G:(b + 1) * G].rearrange("p (gh gl) -> p gl gh", gh=2)
                        .unsqueeze(3).broadcast_to([128, 8, 2, DI]),
                    )

        # ---- t matmul: 2-batch packed, K=128, accumulate over g ----
        tps = [None, None]
        for P in range(2):
            tps[P] = psum.tile([128, 2 * DI], F32, tag="tps", name=f"tps{P}")
            srcv = stripview(xsb if it == 0 else xzb)
            for g in range(16):
                gl, gh = g % 8, g // 8
                rhs = srcv[:, P, gl, :, gh, :DI]  # [p, bv(2), i(16)]
                if it == 0:
                    lhsT = ones128[:]
                else:
                    lhsT = e_sb[:, (P * G + g) * 128:(P * G + g + 1) * 128]
                nc.tensor.matmul(
                    tps[P][:],
                    lhsT=lhsT,
                    rhs=rhs,
                    start=(g == 0),
                    stop=(g == G - 1),
                    skip_group_check=True,
                )

        # ---- small stage, both P merged ----
        # t_sb [128=(bl,o), (P, i)]: diagonal blocks of tps
        t_sb = small.tile([128, 2 * DI], F32, tag="t_sb")
        for P in range(2):
            nc.scalar.copy(t_sb[0:64, P * DI:(P + 1) * DI], tps[P][0:64, 0:DI])
            nc.scalar.copy(t_sb[64:128, P * DI:(P + 1) * DI], tps[P][64:128, DI:2 * DI])

        n2t = small.tile([128, 2], F32, tag="n2t")
        if it < 2:
            qm = small.tile([128, 2 * DI * DI], F32, tag="qm")
            nc.vector.tensor_mul(
                qm[:].rearrange("p (P i j) -> p P i j", P=2, i=DI),
                m2_sb[:].rearrange("p (i j) -> p i j", j=DI)
                .unsqueeze(1).broadcast_to([128, 2, DI, DI]),
                t_sb[:].rearrange("p (P j) -> p P j", P=2)
                .unsqueeze(2).broadcast_to([128, 2, DI, DI]),
            )
            q = small.tile([128, 2 * DI], F32, tag="q")
            nc.vector.reduce_sum(
                q[:], qm[:].rearrange("p (P i j) -> p (P i) j", P=2, i=DI),
                axis=mybir.AxisListType.X,
            )
            scr = small.tile([128, 2 * DI], F32, tag="scr")
            nc.vector.tensor_mul(scr[:], t_sb[:], q[:])
            nc.vector.reduce_sum(
                n2t[:], scr[:].rearrange("p (P i) -> p P i", P=2), axis=mybir.AxisListType.X
            )
        else:
            sm = small.tile([128, 2 * D * DI], F32, tag="sm")
            nc.vector.tensor_mul(
                sm[:].rearrange("p (P d j) -> p P d j", P=2, d=D),
                w_sb[:].rearrange("p (d j) -> p d j", j=DI)
                .unsqueeze(1).broadcast_to([128, 2, D, DI]),
                t_sb[:].rearrange("p (P j) -> p P j", P=2)
                .unsqueeze(2).broadcast_to([128, 2, D, DI]),
            )
            s_sb = small.tile([128, 2 * D], F32, tag="s_sb")
            nc.vector.reduce_sum(
                s_sb[:],
                sm[:].rearrange("p (P d j) -> p (P d) j", P=2, d=D),
                axis=mybir.AxisListType.X,
            )
            scr2 = small.tile([128, 2 * D], F32, tag="scr2")
            nc.vector.tensor_mul(scr2[:], s_sb[:], s_sb[:])
            nc.vector.reduce_sum(
                n2t[:], scr2[:].rearrange("p (P d) -> p P d", P=2), axis=mybir.AxisListType.X
            )

        nt = small.tile([128, 2], F32, tag="nt")
        nc.scalar.activation(nt[:], n2t[:], AF.Ln)
        nc.scalar.activation(nt[:], nt[:], AF.Exp, scale=0.5)
        nc.vector.tensor_scalar_add(nt[:], nt[:], 1e-8)
        dent = small.tile([128, 2], F32, tag="dent")
        nc.vector.scalar_tensor_tensor(dent[:], n2t[:], EPS, nt[:], op0=ALU.add, op1=ALU.mult)
        nc.vector.reciprocal(dent[:], dent[:])
        h = small.tile([128, 2], F32, tag="h")
        nc.vector.tensor_mul(h[:], n2t[:], dent[:])

        if it < 2:
            for P in range(2):
                if it == 0:
                    nc.vector.tensor_scalar_mul(wv0f[P][:], q[:, P * DI:(P + 1) * DI], h[:, P:P + 1])
                    nc.vector.tensor_scalar_mul(wv_bf[P][:, :DI], q[:, P * DI:(P + 1) * DI], h[:, P:P + 1])
                else:
                    nc.vector.scalar_tensor_tensor(
                        wv_bf[P][:, :DI], q[:, P * DI:(P + 1) * DI], h[:, P:P + 1],
                        wv0f[P][:], op0=ALU.mult, op1=ALU.add,
                    )
                trp = psum1.tile([128, 128], BF16, tag="trp")
                for r4 in range(4):
                    nc.tensor.transpose(
                        trp[r4 * 32:(r4 + 1) * 32, :],
                        wv_bf[P][:, :],
                        id_sb[:],
                        tile_position=(0, r4 * 32),
                    )
                tnew = small.tile([128, 128], BF16, tag="trc")
                nc.scalar.copy(tnew[:], trp[:])
                trc[P] = tnew
        else:
            v_sb = small.tile([128, 2 * D], F32, tag="v_sb")
            nc.vector.tensor_mul(
                v_sb[:].rearrange("p (P d) -> p P d", P=2),
                s_sb[:].rearrange("p (P d) -> p P d", P=2),
                h[:].unsqueeze(2).broadcast_to([128, 2, D]),
            )
            for P in range(2):
                nc.sync.dma_start(
                    vout[2 * P:2 * P + 2].rearrange("b o d -> (b o) d"),
                    v_sb[:, P * D:(P + 1) * D],
                )
    ctx.close()


_CACHE = {}


def _get_module():
    if "nc" not in _CACHE:
        nc = bacc.Bacc("TRN2", target_bir_lowering=False, debug=False,
                       enable_asserts=False, num_devices=N_CORES)
        with tile.TileContext(nc) as tc:
            build_kernel(nc, tc)
        nc.compile()
        _CACHE["nc"] = nc
    return _CACHE["nc"]


def _host_inputs(input_vectors, weight_matrix):
    W0 = np.asarray(weight_matrix, dtype=np.float32)[0]          # [O, D, DI]
    M2 = np.einsum("odi,odj->oij", W0, W0).astype(np.float32)    # [O, DI, DI]
    wrep = np.tile(W0.reshape(O, D * DI), (2, 1)).astype(np.float32)
    m2rep = np.tile(M2.reshape(O, DI * DI), (2, 1)).astype(np.float32)
    ident = np.eye(128, dtype=ml_dtypes.bfloat16)
    x = np.ascontiguousarray(np.asarray(input_vectors, dtype=np.float32))
    in_maps = []
    for c in range(N_CORES):
        in_maps.append({
            "x": np.ascontiguousarray(x[c * B:(c + 1) * B]),
            "wrep": wrep,
            "m2rep": m2rep,
            "ident": ident,
        })
    return in_maps


def run(input_vectors, weight_matrix, trace=False, tmpdir=None):
    nc = _get_module()
    in_maps = _host_inputs(input_vectors, weight_matrix)
    res = run_bass_kernel_spmd(
        nc, in_maps, core_ids=list(range(N_CORES)), trace=trace, tmpdir=tmpdir
    )
    out = np.concatenate([res.results[c]["vout"] for c in range(N_CORES)], axis=0)
    return out.astype(np.float32), res


def kernel(input_vectors, weight_matrix):
    out, _ = run(input_vectors, weight_matrix, trace=False)
    return out


# revision 16
# speedup vs baseline: 1.2798x; 1.2798x over previous
"""CapsuleLayer (dynamic routing, 3 iterations) Trainium2 Bass kernel. v2

Full inputs:  input_vectors [32, 2048, 16] f32, weight_matrix [1, 64, 32, 16] f32
Full output:  [32, 64, 32] f32

Sharding: data-parallel over batch; each of 8 NeuronCores processes 4 batches.
weight-derived constants are replicated. No collectives.

Algorithm restructuring (never materializes u = [B,N,O,D] = 537MB):
  xs       = squash(x)                       (per-row scale g = n2/((eps+n2)(1e-8+n)))
  iter 0:  c uniform -> t0[o,i] = (1/64) sum_n xs[n,i]        (ones matmul)
  iter k:  logits = xs @ wv_sum.T            (bf16 matmul, K=16, row-tiled quads)
           e = exp(logits); Z = sum_o e; xz = xs / Z
           t[o,i] = sum_n e[n,o] * xz[n,i]   (bf16 matmul, K=128, 2-batch packed)
  wv      = h * (M2 @ t),  M2 = W^T W (host-precomputed Gram),  h = squash scale of s
            (uses n2 = ||s||^2 = t . (M2 @ t) so s itself is only built at the end)
  output  v = h * (W @ t)  at the last iteration.
Iteration 2 logits use rhs wv0+wv1 (linearity) so no cross-iteration PSUM state.

v2 performance changes vs v1:
  - n-mapping n = p*16 + j (was g*128 + p): input DMA descriptors become 1KB
    contiguous chunks instead of 64B (the indexing of every on-chip array is
    identical under the relabel g<->j; only the input DMA rearrange changes).
  - sqrt computed as exp(0.5*ln(n2)): every scalar-engine function used
    (Exp/Ln/Square/Copy) lives in the single natural_log_exp_and_others ACT
    table set -> no mid-kernel ~2.7us ACT_TABLE_LOAD swaps.
  - squash scale multiply writes the padded/permuted bf16 layout (xsb == old
    xsp) directly; xs f32 and the 4 repack copies are gone.
  - all 16 [128,128] SBUF transposes issued as ONE dma_start_transpose with a
    3D out AP (16 serialized Sync-queue instructions -> 1).
  - e, xz in bf16 (2x DVE read rate, FWL weight loads); t-matmul packs 2
    batches per instruction pair (lhsT [128,128] = [e_b0|e_b1]) halving the
    instruction count; off-diagonal quadrants of the [128,32] PSUM out are
    ignored.
  - small stage (q, n2, h, wv) runs once on both batch-pairs merged [128,32].
  - HAM warmup: a burst of dummy matmuls during the input DMA/squash phase
    keeps the PE clock at 2.4 GHz for the real work.
"""

import os

os.environ.setdefault("MYCRO_LOCAL_CACHE", "1")

import numpy as np
import ml_dtypes

import concourse.bass as bass
import concourse.tile as tile
from concourse import bacc, mybir
from concourse.bass_utils import run_bass_kernel_spmd

AF = mybir.ActivationFunctionType
ALU = mybir.AluOpType
F32 = mybir.dt.float32
BF16 = mybir.dt.bfloat16

N_CORES = 8
B = 4          # batches per core
N = 2048       # input capsules
O = 64         # output capsules
DI = 16        # input capsule dim
D = 32         # output capsule dim
G = 16         # n-groups per batch (n = p*16 + g)
EPS = 0.5

NWARM = int(os.environ.get("CAPS_NWARM", "40"))


def _strip(b, g):
    """(row_base, col_base) of the xsT strip for (batch b, n-group g).

    Quad layout: 4 concurrent K=16 agreement matmuls sit at row groups
    0/32/64/96 = (b%2)*64 + (g//8)*32; col block = ((b//2)*8 + (g%8))*128.
    """
    r = (b % 2) * 64 + (g // 8) * 32
    c = ((b // 2) * 8 + (g % 8)) * 128
    return r, c


def build_kernel(nc: bass.Bass, tc: tile.TileContext):
    from contextlib import ExitStack
    ctx = ExitStack()
    x = nc.dram_tensor("x", [B, N, DI], F32, kind="ExternalInput").ap()
    wrep = nc.dram_tensor("wrep", [128, D * DI], F32, kind="ExternalInput").ap()
    m2rep = nc.dram_tensor("m2rep", [128, DI * DI], F32, kind="ExternalInput").ap()
    ident = nc.dram_tensor("ident", [128, 128], BF16, kind="ExternalInput").ap()
    vout = nc.dram_tensor("vout", [B, O, D], F32, kind="ExternalOutput").ap()

    const = ctx.enter_context(tc.tile_pool(name="const", bufs=1))
    big = ctx.enter_context(tc.tile_pool(name="big", bufs=1))
    small = ctx.enter_context(tc.tile_pool(name="small", bufs=2))
    psum = ctx.enter_context(tc.tile_pool(name="psum", bufs=2, space="PSUM"))
    psum1 = ctx.enter_context(tc.tile_pool(name="psum1", bufs=1, space="PSUM"))

    # ---- constants ----
    w_sb = const.tile([128, D * DI], F32, tag="w_sb")
    m2_sb = const.tile([128, DI * DI], F32, tag="m2_sb")
    id_sb = const.tile([128, 128], BF16, tag="id_sb")
    ones128 = const.tile([128, 128], BF16, tag="ones128")
    nc.sync.dma_start(w_sb[:], wrep)
    nc.sync.dma_start(m2_sb[:], m2rep)
    nc.sync.dma_start(id_sb[:], ident)
    nc.gpsimd.memset(ones128[:], 1.0 / O)

    # persistent wv tiles: [128=(bl,o), 32=(i|pad)] pads stay zero forever
    wv_bf = [const.tile([128, 2 * DI], BF16, tag=f"wv_bf{P}", name=f"wv_bf{P}") for P in range(2)]
    for P in range(2):
        nc.gpsimd.memset(wv_bf[P][:], 0.0)
    wv0f = [const.tile([128, DI], F32, tag=f"wv0f_{P}", name=f"wv0f_{P}") for P in range(2)]

    # xsb: squashed input, bf16, padded strip layout
    #   col = block*128 + (b%2)*64 + (g//8)*32 + i,  block = (b//2)*8 + (g%8)
    # (cols 16..31 of each 32-strip are pad; memset once, early)
    xsb = big.tile([128, 2048], BF16, tag="xsb")
    nc.gpsimd.memset(xsb[:], 0.0)

    # ---- HAM warmup: dummy matmuls while DMA/squash run ----
    if NWARM:
        warm = psum1.tile([128, O], F32, tag="warm")
        for _ in range(NWARM):
            nc.tensor.matmul(warm[:], lhsT=id_sb[:], rhs=id_sb[:, :O],
                             start=True, stop=True)

    # ---- load x:  xr [128, (b, g, i)] with n = p*16 + g ----
    xr = big.tile([128, B * G * DI], F32, tag="xr")
    nc.sync.dma_start(
        xr[:].rearrange("p (b g i) -> p b g i", b=B, g=G),
        x.rearrange("b (p g) i -> p b g i", g=G),
    )

    # ---- squash ----
    xsq = big.tile([128, B * G * DI], F32, tag="xsq")
    nc.vector.tensor_mul(xsq[:], xr[:], xr[:])
    n2x = small.tile([128, B * G], F32, tag="n2x")
    nc.vector.reduce_sum(n2x[:], xsq[:].rearrange("p (r i) -> p r i", i=DI), axis=mybir.AxisListType.X)
    # n = sqrt(n2) = exp(0.5*ln(n2)); keeps scalar engine on one table set
    nx = small.tile([128, B * G], F32, tag="nx")
    nc.scalar.activation(nx[:], n2x[:], AF.Ln)
    nc.scalar.activation(nx[:], nx[:], AF.Exp, scale=0.5)
    nc.vector.tensor_scalar_add(nx[:], nx[:], 1e-8)
    denx = small.tile([128, B * G], F32, tag="denx")
    nc.vector.scalar_tensor_tensor(denx[:], n2x[:], EPS, nx[:], op0=ALU.add, op1=ALU.mult)
    nc.vector.reciprocal(denx[:], denx[:])
    gx = small.tile([128, B * G], F32, tag="gx")
    nc.vector.tensor_mul(gx[:], n2x[:], denx[:])

    # xsb views: per-batch strided [p, gl(8), gh(2), i(16)]
    def stripview(t):
        return t[:].rearrange("p (P gl bv gh c) -> p P gl bv gh c", P=2, gl=8, bv=2, gh=2)

    xsbv = stripview(xsb)
    for b in range(B):
        eng = nc.vector if b < 2 else nc.gpsimd
        eng.tensor_mul(
            xsbv[:, b // 2, :, b % 2, :, :DI],
            xr[:, b * G * DI:(b + 1) * G * DI].rearrange(
                "p (gh gl i) -> p gl gh i", gh=2, gl=8
            ),
            gx[:, b * G:(b + 1) * G].rearrange("p (gh gl) -> p gl gh", gh=2)
            .unsqueeze(3).broadcast_to([128, 8, 2, DI]),
        )

    # ---- xsT: all 16 block transposes in ONE instruction ----
    xsT = big.tile([128, 2048], BF16, tag="xsT")
    nc.sync.dma_start(
        xsT[:].rearrange("p (c f) -> p c f", f=128),
        xsb[:],
        transpose=True,
    )

    # ---- persistent state ----
    # e_sb cols: (P, g, bl, o)  -> lhsT pair = e_sb[:, (P*16+g)*128 : +128]
    e_sb = big.tile([128, 2 * G * 2 * O], BF16, tag="e_sb")
    rz = small.tile([128, B * G], F32, tag="rz")
    # xzb: same padded strip layout as xsb (pads never read)
    xzb = big.tile([128, 2048], BF16, tag="xzb")
    trc = [None, None]

    for it in range(3):
        if it > 0:
            for P in range(2):
                L = [psum.tile([128, G * O], F32, tag="logits", name=f"L{bv}") for bv in range(2)]
                # (bv, gh) vary fastest -> consecutive matmuls hit the 4
                # distinct PE row groups -> 4-way quad concurrency
                for gl in range(8):
                    for bv in range(2):
                        for gh in range(2):
                            b = 2 * P + bv
                            g = gh * 8 + gl
                            r, c = _strip(b, g)
                            nc.tensor.matmul(
                                L[bv][:, g * O:(g + 1) * O],
                                lhsT=xsT[r:r + DI, c:c + 128],
                                rhs=trc[P][r:r + DI, bv * O:(bv + 1) * O],
                                tile_position=(r, 0),
                                start=True,
                                stop=True,
                            )
                for bv in range(2):
                    b = 2 * P + bv
                    # e (bf16) strided into (P, g, bl, o) layout
                    ev = e_sb[:].rearrange(
                        "p (Pp g two o) -> p Pp g two o", Pp=2, g=G, two=2
                    )[:, P, :, bv, :]
                    nc.scalar.activation(ev, L[bv][:].rearrange("p (g o) -> p g o", o=O), AF.Exp)
                    zb = small.tile([128, G], F32, tag="zb")
                    nc.vector.reduce_sum(zb[:], ev, axis=mybir.AxisListType.X)
                    nc.vector.reciprocal(rz[:, b * G:(b + 1) * G], zb[:])
                    nc.gpsimd.tensor_mul(
                        stripview(xzb)[:, b // 2, :, b % 2, :, :DI],
                        xsbv[:, b // 2, :, b % 2, :, :DI],
                        rz[:, b * G:(b + 1) * G].rearrange("p (gh gl) -> p gl gh", gh=2)
                        .unsqueeze(3).broadcast_to([128, 8, 2, DI]),
                    )

        # ---- t matmul: 2-batch packed, K=128, accumulate over g ----
        tps = [None, None]
        for P in range(2):
            tps[P] = psum.tile([128, 2 * DI], F32, tag="tps", name=f"tps{P}")
            srcv = stripview(xsb if it == 0 else xzb)
            for g in range(16):
                gl, gh = g % 8, g // 8
                rhs = srcv[:, P, gl, :, gh, :DI]  # [p, bv(2), i(16)]
                if it == 0:
                    lhsT = ones128[:]
                else:
                    lhsT = e_sb[:, (P * G + g) * 128:(P * G + g + 1) * 128]
                nc.tensor.matmul(
                    tps[P][:],
                    lhsT=lhsT,
                    rhs=rhs,
                    start=(g == 0),
                    stop=(g == G - 1),
                    skip_group_check=True,
                )

        # ---- small stage, both P merged ----
        # t_sb [128=(bl,o), (P, i)]: diagonal blocks of tps
        t_sb = small.tile([128, 2 * DI], F32, tag="t_sb")
        for P in range(2):
            nc.vector.tensor_copy(t_sb[0:64, P * DI:(P + 1) * DI], tps[P][0:64, 0:DI])
            nc.vector.tensor_copy(t_sb[64:128, P * DI:(P + 1) * DI], tps[P][64:128, DI:2 * DI])

        n2t = small.tile([128, 2], F32, tag="n2t")
        if it < 2:
            qm = small.tile([128, 2 * DI * DI], F32, tag="qm")
            nc.vector.tensor_mul(
                qm[:].rearrange("p (P i j) -> p P i j", P=2, i=DI),
                m2_sb[:].rearrange("p (i j) -> p i j", j=DI)
                .unsqueeze(1).broadcast_to([128, 2, DI, DI]),
                t_sb[:].rearrange("p (P j) -> p P j", P=2)
                .unsqueeze(2).broadcast_to([128, 2, DI, DI]),
            )
            q = small.tile([128, 2 * DI], F32, tag="q")
            nc.vector.reduce_sum(
                q[:], qm[:].rearrange("p (P i j) -> p (P i) j", P=2, i=DI),
                axis=mybir.AxisListType.X,
            )
            scr = small.tile([128, 2 * DI], F32, tag="scr")
            nc.vector.tensor_mul(scr[:], t_sb[:], q[:])
            nc.vector.reduce_sum(
                n2t[:], scr[:].rearrange("p (P i) -> p P i", P=2), axis=mybir.AxisListType.X
            )
        else:
            sm = small.tile([128, 2 * D * DI], F32, tag="sm")
            nc.vector.tensor_mul(
                sm[:].rearrange("p (P d j) -> p P d j", P=2, d=D),
                w_sb[:].rearrange("p (d j) -> p d j", j=DI)
                .unsqueeze(1).broadcast_to([128, 2, D, DI]),
                t_sb[:].rearrange("p (P j) -> p P j", P=2)
                .unsqueeze(2).broadcast_to([128, 2, D, DI]),
            )
            s_sb = small.tile([128, 2 * D], F32, tag="s_sb")
            nc.vector.reduce_sum(
                s_sb[:],
                sm[:].rearrange("p (P d j) -> p (P d) j", P=2, d=D),
                axis=mybir.AxisListType.X,
            )
            scr2 = small.tile([128, 2 * D], F32, tag="scr2")
            nc.vector.tensor_mul(scr2[:], s_sb[:], s_sb[:])
            nc.vector.reduce_sum(
                n2t[:], scr2[:].rearrange("p (P d) -> p P d", P=2), axis=mybir.AxisListType.X
            )

        nt = small.tile([128, 2], F32, tag="nt")
        nc.scalar.activation(nt[:], n2t[:], AF.Ln)
        nc.scalar.activation(nt[:], nt[:], AF.Exp, scale=0.5)
        nc.vector.tensor_scalar_add(nt[:], nt[:], 1e-8)
        dent = small.tile([128, 2], F32, tag="dent")
        nc.vector.scalar_tensor_tensor(dent[:], n2t[:], EPS, nt[:], op0=ALU.add, op1=ALU.mult)
        nc.vector.reciprocal(dent[:], dent[:])
        h = small.tile([128, 2], F32, tag="h")
        nc.vector.tensor_mul(h[:], n2t[:], dent[:])

        if it < 2:
            for P in range(2):
                if it == 0:
                    nc.vector.tensor_scalar_mul(wv0f[P][:], q[:, P * DI:(P + 1) * DI], h[:, P:P + 1])
                    nc.vector.tensor_scalar_mul(wv_bf[P][:, :DI], q[:, P * DI:(P + 1) * DI], h[:, P:P + 1])
                else:
                    nc.vector.scalar_tensor_tensor(
                        wv_bf[P][:, :DI], q[:, P * DI:(P + 1) * DI], h[:, P:P + 1],
                        wv0f[P][:], op0=ALU.mult, op1=ALU.add,
                    )
                trp = psum1.tile([128, 128], BF16, tag="trp")
                for r4 in range(4):
                    nc.tensor.transpose(
                        trp[r4 * 32:(r4 + 1) * 32, :],
                        wv_bf[P][:, :],
                        id_sb[:],
                        tile_position=(0, r4 * 32),
                    )
                tnew = small.tile([128, 128], BF16, tag="trc")
                nc.vector.tensor_copy(tnew[:], trp[:])
                trc[P] = tnew
        else:
            v_sb = small.tile([128, 2 * D], F32, tag="v_sb")
            nc.vector.tensor_mul(
                v_sb[:].rearrange("p (P d) -> p P d", P=2),
                s_sb[:].rearrange("p (P d) -> p P d", P=2),
                h[:].unsqueeze(2).broadcast_to([128, 2, D]),
            )
            for P in range(2):
                nc.sync.dma_start(
                    vout[2 * P:2 * P + 2].rearrange("b o d -> (b o) d"),
                    v_sb[:, P * D:(P + 1) * D],
                )
    ctx.close()


_CACHE = {}


class _OneActSetBacc(bacc.Bacc):
    """Bacc whose act-table pass sees only natural_log_exp_and_others.

    The stock pass picks the first table set containing each activation
    function (exp -> exp_and_others, ln -> natural_log), which inserts a
    ~2.7us ACT_TABLE_LOAD swap at every exp<->ln transition.  This kernel
    only uses {exp, ln, copy, square}, all present in the single set
    natural_log_exp_and_others, so blank out every other set (keeping list
    positions, since act_func_set_id is the index into act_info.json).
    """

    def insert_act_table_loads(self):
        import bass_rust as _bass_rust
        from concourse.hw_specs import get_activation_tables

        has_activation = any(
            isinstance(i, mybir.InstActivation)
            for b in self.main_func.blocks
            for i in b.instructions
        )
        if not has_activation:
            return
        tables = []
        for name, fns in get_activation_tables(self.m.arch).items():
            if name == "natural_log_exp_and_others":
                tables.append((name, fns))
            else:
                tables.append((name, set()))
        _bass_rust.insert_act_table_loads(self, tables)


def _get_module():
    if "nc" not in _CACHE:
        nc = _OneActSetBacc("TRN2", target_bir_lowering=False, debug=False,
                            enable_asserts=False, num_devices=N_CORES)
        with tile.TileContext(nc) as tc:
            build_kernel(nc, tc)
        nc.compile()
        _CACHE["nc"] = nc
    return _CACHE["nc"]


def _host_inputs(input_vectors, weight_matrix):
    W0 = np.asarray(weight_matrix, dtype=np.float32)[0]          # [O, D, DI]
    M2 = np.einsum("odi,odj->oij", W0, W0).astype(np.float32)    # [O, DI, DI]
    wrep = np.tile(W0.reshape(O, D * DI), (2, 1)).astype(np.float32)
    m2rep = np.tile(M2.reshape(O, DI * DI), (2, 1)).astype(np.float32)
    ident = np.eye(128, dtype=ml_dtypes.bfloat16)
    x = np.ascontiguousarray(np.asarray(input_vectors, dtype=np.float32))
    in_maps = []
    for c in range(N_CORES):
        in_maps.append({
            "x": np.ascontiguousarray(x[c * B:(c + 1) * B]),
            "wrep": wrep,
            "m2rep": m2rep,
            "ident": ident,
        })
    return in_maps


def run(input_vectors, weight_matrix, trace=False, tmpdir=None):
    nc = _get_module()
    in_maps = _host_inputs(input_vectors, weight_matrix)
    res = run_bass_kernel_spmd(
        nc, in_maps, core_ids=list(range(N_CORES)), trace=trace, tmpdir=tmpdir
    )
    out = np.concatenate([res.results[c]["vout"] for c in range(N_CORES)], axis=0)
    return out.astype(np.float32), res


def kernel(input_vectors, weight_matrix):
    out, _ = run(input_vectors, weight_matrix, trace=False)
    return out


# revision 21
# speedup vs baseline: 1.3528x; 1.0571x over previous
"""CapsuleLayer (dynamic routing, 3 iterations) Trainium2 Bass kernel. v2

Full inputs:  input_vectors [32, 2048, 16] f32, weight_matrix [1, 64, 32, 16] f32
Full output:  [32, 64, 32] f32

Sharding: data-parallel over batch; each of 8 NeuronCores processes 4 batches.
weight-derived constants are replicated. No collectives.

Algorithm restructuring (never materializes u = [B,N,O,D] = 537MB):
  xs       = squash(x)                       (per-row scale g = n2/((eps+n2)(1e-8+n)))
  iter 0:  c uniform -> t0[o,i] = (1/64) sum_n xs[n,i]        (ones matmul)
  iter k:  logits = xs @ wv_sum.T            (bf16 matmul, K=16, row-tiled quads)
           e = exp(logits); Z = sum_o e; xz = xs / Z
           t[o,i] = sum_n e[n,o] * xz[n,i]   (bf16 matmul, K=128, 2-batch packed)
  wv      = h * (M2 @ t),  M2 = W^T W (host-precomputed Gram),  h = squash scale of s
            (uses n2 = ||s||^2 = t . (M2 @ t) so s itself is only built at the end)
  output  v = h * (W @ t)  at the last iteration.
Iteration 2 logits use rhs wv0+wv1 (linearity) so no cross-iteration PSUM state.

v2 performance changes vs v1:
  - n-mapping n = p*16 + j (was g*128 + p): input DMA descriptors become 1KB
    contiguous chunks instead of 64B (the indexing of every on-chip array is
    identical under the relabel g<->j; only the input DMA rearrange changes).
  - sqrt computed as exp(0.5*ln(n2)): every scalar-engine function used
    (Exp/Ln/Square/Copy) lives in the single natural_log_exp_and_others ACT
    table set -> no mid-kernel ~2.7us ACT_TABLE_LOAD swaps.
  - squash scale multiply writes the padded/permuted bf16 layout (xsb == old
    xsp) directly; xs f32 and the 4 repack copies are gone.
  - all 16 [128,128] SBUF transposes issued as ONE dma_start_transpose with a
    3D out AP (16 serialized Sync-queue instructions -> 1).
  - e, xz in bf16 (2x DVE read rate, FWL weight loads); t-matmul packs 2
    batches per instruction pair (lhsT [128,128] = [e_b0|e_b1]) halving the
    instruction count; off-diagonal quadrants of the [128,32] PSUM out are
    ignored.
  - small stage (q, n2, h, wv) runs once on both batch-pairs merged [128,32].
  - HAM warmup: a burst of dummy matmuls during the input DMA/squash phase
    keeps the PE clock at 2.4 GHz for the real work.
"""

import os

os.environ.setdefault("MYCRO_LOCAL_CACHE", "1")

import numpy as np
import ml_dtypes

import concourse.bass as bass
import concourse.tile as tile
from concourse import bacc, mybir
from concourse.bass_utils import run_bass_kernel_spmd

AF = mybir.ActivationFunctionType
ALU = mybir.AluOpType
F32 = mybir.dt.float32
BF16 = mybir.dt.bfloat16

N_CORES = 8
B = 4          # batches per core
N = 2048       # input capsules
O = 64         # output capsules
DI = 16        # input capsule dim
D = 32         # output capsule dim
G = 16         # n-groups per batch (n = p*16 + g)
EPS = 0.5

NWARM = int(os.environ.get("CAPS_NWARM", "40"))


def _strip(b, g):
    """(row_base, col_base) of the xsT strip for (batch b, n-group g).

    Quad layout: 4 concurrent K=16 agreement matmuls sit at row groups
    0/32/64/96 = (b%2)*64 + (g//8)*32; col block = ((b//2)*8 + (g%8))*128.
    """
    r = (b % 2) * 64 + (g // 8) * 32
    c = ((b // 2) * 8 + (g % 8)) * 128
    return r, c


def build_kernel(nc: bass.Bass, tc: tile.TileContext):
    from contextlib import ExitStack
    ctx = ExitStack()
    x = nc.dram_tensor("x", [B, N, DI], F32, kind="ExternalInput").ap()
    wrep = nc.dram_tensor("wrep", [128, D * DI], F32, kind="ExternalInput").ap()
    m2rep = nc.dram_tensor("m2rep", [128, DI * DI], F32, kind="ExternalInput").ap()
    ident = nc.dram_tensor("ident", [128, 128], BF16, kind="ExternalInput").ap()
    vout = nc.dram_tensor("vout", [B, O, D], F32, kind="ExternalOutput").ap()

    const = ctx.enter_context(tc.tile_pool(name="const", bufs=1))
    big = ctx.enter_context(tc.tile_pool(name="big", bufs=1))
    small = ctx.enter_context(tc.tile_pool(name="small", bufs=2))
    psum = ctx.enter_context(tc.tile_pool(name="psum", bufs=2, space="PSUM"))
    psum1 = ctx.enter_context(tc.tile_pool(name="psum1", bufs=1, space="PSUM"))

    # ---- constants (ident first: warmup matmuls depend on it) ----
    w_sb = const.tile([128, D * DI], F32, tag="w_sb")
    m2_sb = const.tile([128, DI * DI], F32, tag="m2_sb")
    id_sb = const.tile([128, 128], BF16, tag="id_sb")
    ones128 = const.tile([128, 128], BF16, tag="ones128")
    nc.sync.dma_start(id_sb[:], ident)
    nc.gpsimd.memset(ones128[:], 1.0 / O)

    # persistent wv tiles: [128=(bl,o), 32=(i|pad)] pads stay zero forever
    wv_bf = [const.tile([128, 2 * DI], BF16, tag=f"wv_bf{P}", name=f"wv_bf{P}") for P in range(2)]
    for P in range(2):
        nc.gpsimd.memset(wv_bf[P][:], 0.0)
    wv0f = [const.tile([128, DI], F32, tag=f"wv0f_{P}", name=f"wv0f_{P}") for P in range(2)]

    # xsb: squashed input, bf16, padded strip layout
    #   col = block*128 + (b%2)*64 + (g//8)*32 + i,  block = (b//2)*8 + (g%8)
    # (cols 16..31 of each 32-strip are pad; memset once, early)
    xsb = big.tile([128, 2048], BF16, tag="xsb")
    nc.gpsimd.memset(xsb[:], 0.0)

    # ---- HAM warmup: dummy matmuls while DMA/squash run ----
    if NWARM:
        warm = psum1.tile([128, O], F32, tag="warm")
        for _ in range(NWARM):
            nc.tensor.matmul(warm[:], lhsT=id_sb[:], rhs=id_sb[:, :O],
                             start=True, stop=True)

    # ---- load x:  xr [128, (b, g, i)] with n = p*16 + g ----
    xr = big.tile([128, B * G * DI], F32, tag="xr")
    nc.sync.dma_start(
        xr[:].rearrange("p (b g i) -> p b g i", b=B, g=G),
        x.rearrange("b (p g) i -> p b g i", g=G),
    )
    nc.sync.dma_start(w_sb[:], wrep)
    nc.sync.dma_start(m2_sb[:], m2rep)

    # ---- squash ----
    xsq = big.tile([128, B * G * DI], F32, tag="xsq")
    nc.scalar.activation(xsq[:], xr[:], AF.Square)
    n2x = small.tile([128, B * G], F32, tag="n2x")
    nc.vector.reduce_sum(n2x[:], xsq[:].rearrange("p (r i) -> p r i", i=DI), axis=mybir.AxisListType.X)
    # n = sqrt(n2) = exp(0.5*ln(n2)); keeps scalar engine on one table set
    nx = small.tile([128, B * G], F32, tag="nx")
    nc.scalar.activation(nx[:], n2x[:], AF.Ln)
    nc.scalar.activation(nx[:], nx[:], AF.Exp, scale=0.5)
    nc.vector.tensor_scalar_add(nx[:], nx[:], 1e-8)
    denx = small.tile([128, B * G], F32, tag="denx")
    nc.vector.scalar_tensor_tensor(denx[:], n2x[:], EPS, nx[:], op0=ALU.add, op1=ALU.mult)
    nc.vector.reciprocal(denx[:], denx[:])
    gx = small.tile([128, B * G], F32, tag="gx")
    nc.vector.tensor_mul(gx[:], n2x[:], denx[:])

    # xsb views: per-batch strided [p, gl(8), gh(2), i(16)]
    def stripview(t):
        return t[:].rearrange("p (P gl bv gh c) -> p P gl bv gh c", P=2, gl=8, bv=2, gh=2)

    # ---- second warmup burst, gated on gx so it spans the squash tail ----
    if NWARM:
        warm_rhs = small.tile([128, O], BF16, tag="warm_rhs")
        nc.vector.tensor_copy(warm_rhs[:], gx[:, :O])
        for _ in range(NWARM):
            nc.tensor.matmul(warm[:], lhsT=id_sb[:], rhs=warm_rhs[:],
                             start=True, stop=True)

    xsbv = stripview(xsb)
    for b in range(B):
        eng = nc.vector if b < 2 else nc.gpsimd
        eng.tensor_mul(
            xsbv[:, b // 2, :, b % 2, :, :DI],
            xr[:, b * G * DI:(b + 1) * G * DI].rearrange(
                "p (gh gl i) -> p gl gh i", gh=2, gl=8
            ),
            gx[:, b * G:(b + 1) * G].rearrange("p (gh gl) -> p gl gh", gh=2)
            .unsqueeze(3).broadcast_to([128, 8, 2, DI]),
        )

    # ---- xsT: all 16 block transposes in ONE instruction ----
    xsT = big.tile([128, 2048], BF16, tag="xsT")
    nc.sync.dma_start(
        xsT[:].rearrange("p (c f) -> p c f", f=128),
        xsb[:],
        transpose=True,
    )

    # ---- persistent state ----
    # e_sb cols: (P, g, bl, o)  -> lhsT pair = e_sb[:, (P*16+g)*128 : +128]
    e_sb = big.tile([128, 2 * G * 2 * O], BF16, tag="e_sb")
    rz = small.tile([128, B * G], F32, tag="rz")
    # xzb: same padded strip layout as xsb (pads never read)
    xzb = big.tile([128, 2048], BF16, tag="xzb")
    trc = [None, None]

    for it in range(3):
        if it > 0:
            for P in range(2):
                L = [psum.tile([128, G * O], F32, tag="logits", name=f"L{bv}") for bv in range(2)]
                # (bv, gh) vary fastest -> consecutive matmuls hit the 4
                # distinct PE row groups -> 4-way quad concurrency
                for gl in range(8):
                    for bv in range(2):
                        for gh in range(2):
                            b = 2 * P + bv
                            g = gh * 8 + gl
                            r, c = _strip(b, g)
                            nc.tensor.matmul(
                                L[bv][:, g * O:(g + 1) * O],
                                lhsT=xsT[r:r + DI, c:c + 128],
                                rhs=trc[P][r:r + DI, bv * O:(bv + 1) * O],
                                tile_position=(r, 0),
                                start=True,
                                stop=True,
                            )
                for bv in range(2):
                    b = 2 * P + bv
                    # e (bf16) strided into (P, g, bl, o) layout
                    ev = e_sb[:].rearrange(
                        "p (Pp g two o) -> p Pp g two o", Pp=2, g=G, two=2
                    )[:, P, :, bv, :]
                    nc.scalar.activation(ev, L[bv][:].rearrange("p (g o) -> p g o", o=O), AF.Exp)
                    zb = small.tile([128, G], F32, tag="zb")
                    nc.vector.reduce_sum(zb[:], ev, axis=mybir.AxisListType.X)
                    nc.vector.reciprocal(rz[:, b * G:(b + 1) * G], zb[:])
                    nc.gpsimd.tensor_mul(
                        stripview(xzb)[:, b // 2, :, b % 2, :, :DI],
                        xsbv[:, b // 2, :, b % 2, :, :DI],
                        rz[:, b * G:(b + 1) * G].rearrange("p (gh gl) -> p gl gh", gh=2)
                        .unsqueeze(3).broadcast_to([128, 8, 2, DI]),
                    )

        # ---- t matmul: 2-batch packed, K=128, accumulate over g ----
        tps = [None, None]
        for P in range(2):
            tps[P] = psum.tile([128, 2 * DI], F32, tag="tps", name=f"tps{P}")
            srcv = stripview(xsb if it == 0 else xzb)
            for g in range(16):
                gl, gh = g % 8, g // 8
                rhs = srcv[:, P, gl, :, gh, :DI]  # [p, bv(2), i(16)]
                if it == 0:
                    lhsT = ones128[:]
                else:
                    lhsT = e_sb[:, (P * G + g) * 128:(P * G + g + 1) * 128]
                nc.tensor.matmul(
                    tps[P][:],
                    lhsT=lhsT,
                    rhs=rhs,
                    start=(g == 0),
                    stop=(g == G - 1),
                    skip_group_check=True,
                )

        # ---- small stage, per P (P0's trc unblocks next-iter P0 agreements
        # while P1's chain still runs) ----
        for P in range(2):
            # t_sb [128=(bl,o), i]: diagonal blocks of tps; copies run on two
            # different engines so they land in parallel
            t_sb = small.tile([128, DI], F32, tag=f"t_sb{P}", name=f"t_sb{P}")
            nc.vector.tensor_copy(t_sb[0:64, :], tps[P][0:64, 0:DI])
            nc.scalar.copy(t_sb[64:128, :], tps[P][64:128, DI:2 * DI])

            n2t = small.tile([128, 1], F32, tag=f"n2t{P}", name=f"n2t{P}")
            if it < 2:
                qm = small.tile([128, DI * DI], F32, tag=f"qm{P}", name=f"qm{P}")
                nc.vector.tensor_mul(
                    qm[:].rearrange("p (i j) -> p i j", j=DI),
                    m2_sb[:].rearrange("p (i j) -> p i j", j=DI),
                    t_sb[:].unsqueeze(1).broadcast_to([128, DI, DI]),
                )
                acc = small.tile([128, DI], F32, tag=f"acc{P}", name=f"acc{P}")
                nc.vector.reduce_sum(
                    acc[:], qm[:].rearrange("p (i j) -> p i j", j=DI),
                    axis=mybir.AxisListType.X,
                )
            else:
                sm = small.tile([128, D * DI], F32, tag=f"sm{P}", name=f"sm{P}")
                nc.gpsimd.tensor_mul(
                    sm[:].rearrange("p (d j) -> p d j", j=DI),
                    w_sb[:].rearrange("p (d j) -> p d j", j=DI),
                    t_sb[:].unsqueeze(1).broadcast_to([128, D, DI]),
                )
                acc = small.tile([128, D], F32, tag=f"acc{P}", name=f"acc{P}")
                nc.vector.reduce_sum(
                    acc[:], sm[:].rearrange("p (d j) -> p d j", j=DI),
                    axis=mybir.AxisListType.X,
                )
            # acc = q (it<2) or s (it==2); n2 = sum(acc * ref) fused via stt
            aw = DI if it < 2 else D
            ref = t_sb if it < 2 else acc
            scr = small.tile([128, D], F32, tag=f"scr{P}", name=f"scr{P}")
            nc.vector.scalar_tensor_tensor(
                scr[:, :aw], acc[:], 1.0, ref[:],
                op0=ALU.mult, op1=ALU.mult, accum_out=n2t[:],
            )
            # off-critical-path: n2+eps and acc*n2
            n2e = small.tile([128, 1], F32, tag=f"n2e{P}", name=f"n2e{P}")
            nc.vector.tensor_scalar_add(n2e[:], n2t[:], EPS)
            ha = small.tile([128, D], F32, tag=f"ha{P}", name=f"ha{P}")
            nc.vector.tensor_scalar_mul(ha[:, :aw], acc[:], n2t[:])
            # critical path: n = exp(.5 ln n2); den = (n+1e-8)(n2+eps); 1/den
            nt = small.tile([128, 1], F32, tag=f"nt{P}", name=f"nt{P}")
            nc.scalar.activation(nt[:], n2t[:], AF.Ln)
            nc.scalar.activation(nt[:], nt[:], AF.Exp, scale=0.5)
            den = small.tile([128, 1], F32, tag=f"den{P}", name=f"den{P}")
            nc.vector.scalar_tensor_tensor(den[:], nt[:], 1e-8, n2e[:], op0=ALU.add, op1=ALU.mult)
            nc.vector.reciprocal(den[:], den[:])

            if it < 2:
                # wv = q*h + wv0 = ha*deninv + wv0
                if it == 0:
                    nc.vector.tensor_scalar_mul(wv0f[P][:], ha[:, :DI], den[:])
                    nc.vector.tensor_scalar_mul(wv_bf[P][:, :DI], ha[:, :DI], den[:])
                else:
                    nc.vector.scalar_tensor_tensor(
                        wv_bf[P][:, :DI], ha[:, :DI], den[:],
                        wv0f[P][:], op0=ALU.mult, op1=ALU.add,
                    )
                trp = psum1.tile([128, 128], BF16, tag="trp")
                for r4 in range(4):
                    nc.tensor.transpose(
                        trp[r4 * 32:(r4 + 1) * 32, :],
                        wv_bf[P][:, :],
                        id_sb[:],
                        tile_position=(0, r4 * 32),
                    )
                tnew = small.tile([128, 128], BF16, tag="trc")
                nc.vector.tensor_copy(tnew[:], trp[:])
                trc[P] = tnew
            else:
                v_sb = small.tile([128, D], F32, tag=f"v_sb{P}", name=f"v_sb{P}")
                nc.vector.tensor_scalar_mul(v_sb[:], ha[:, :D], den[:])
                nc.sync.dma_start(
                    vout[2 * P:2 * P + 2].rearrange("b o d -> (b o) d"),
                    v_sb[:],
                )
    ctx.close()


_CACHE = {}


class _OneActSetBacc(bacc.Bacc):
    """Bacc whose act-table pass sees only natural_log_exp_and_others.

    The stock pass picks the first table set containing each activation
    function (exp -> exp_and_others, ln -> natural_log), which inserts a
    ~2.7us ACT_TABLE_LOAD swap at every exp<->ln transition.  This kernel
    only uses {exp, ln, copy, square}, all present in the single set
    natural_log_exp_and_others, so blank out every other set (keeping list
    positions, since act_func_set_id is the index into act_info.json).
    """

    def insert_act_table_loads(self):
        import bass_rust as _bass_rust
        from concourse.hw_specs import get_activation_tables

        has_activation = any(
            isinstance(i, mybir.InstActivation)
            for b in self.main_func.blocks
            for i in b.instructions
        )
        if not has_activation:
            return
        tables = []
        for name, fns in get_activation_tables(self.m.arch).items():
            if name == "natural_log_exp_and_others":
                tables.append((name, fns))
            else:
                tables.append((name, set()))
        _bass_rust.insert_act_table_loads(self, tables)


def _get_module():
    if "nc" not in _CACHE:
        nc = _OneActSetBacc("TRN2", target_bir_lowering=False, debug=False,
                            enable_asserts=False, num_devices=N_CORES)
        with tile.TileContext(nc) as tc:
            build_kernel(nc, tc)
        nc.compile()
        _CACHE["nc"] = nc
    return _CACHE["nc"]


def _host_inputs(input_vectors, weight_matrix):
    W0 = np.asarray(weight_matrix, dtype=np.float32)[0]          # [O, D, DI]
    M2 = np.einsum("odi,odj->oij", W0, W0).astype(np.float32)    # [O, DI, DI]
    wrep = np.tile(W0.reshape(O, D * DI), (2, 1)).astype(np.float32)
    m2rep = np.tile(M2.reshape(O, DI * DI), (2, 1)).astype(np.float32)
    ident = np.eye(128, dtype=ml_dtypes.bfloat16)
    x = np.ascontiguousarray(np.asarray(input_vectors, dtype=np.float32))
    in_maps = []
    for c in range(N_CORES):
        in_maps.append({
            "x": np.ascontiguousarray(x[c * B:(c + 1) * B]),
            "wrep": wrep,
            "m2rep": m2rep,
            "ident": ident,
        })
    return in_maps


def run(input_vectors, weight_matrix, trace=False, tmpdir=None):
    nc = _get_module()
    in_maps = _host_inputs(input_vectors, weight_matrix)
    res = run_bass_kernel_spmd(
        nc, in_maps, core_ids=list(range(N_CORES)), trace=trace, tmpdir=tmpdir
    )
    out = np.concatenate([res.results[c]["vout"] for c in range(N_CORES)], axis=0)
    return out.astype(np.float32), res


def kernel(input_vectors, weight_matrix):
    out, _ = run(input_vectors, weight_matrix, trace=False)
    return out
